# revision 9
# baseline (speedup 1.0000x reference)
"""Trainium2 Bass kernel for nn_ParallelCA: 5 rounds of spatial-softmax
attention + low-rank tanh cross-attention + channel L2 norm.

Data parallel: N=8 samples, one per NeuronCore (8 cores).

Per-core layouts (sample index dropped):
  fpN: [C=256 (2x128 part chunks), HW=1024]   "normal"
  fpT: [HW=1024 (8x128 part chunks), C=256]   "transposed"
  B[j,i] = tanh(sum_l f1cl[l,j] f2cl[l,i]) = A[i,j]   (j on partitions)
  D[i,j] = tanh(sum_l f2cl[l,i] f1cl[l,j]) = A[i,j]   (i on partitions)
  f1_hatT[i,c] = sum_j B[j,i] s1T[j,c]   (matmul lhsT=B, rhs=s1T)
  f2_hatT[i,c] = sum_j D[j,i] s2T[j,c]   (matmul lhsT=D, rhs=s2T)
L2 norm over c is then a free-dim reduction; softmax conv bias cancels.
Tensors feeding the PE are stored as float32r (full-rate fp32 matmul).
"""

import numpy as np

C = 256
HW = 1024
CL = 16
NCORES = 8
PQ = 48  # proj rows at 0..15, conv row at 32 (PSUM partition alignment)
NITER = 5

_CACHE = {}


def _build():
    import concourse.tile as tile
    import concourse.mybir as mybir
    from concourse.bacc import Bacc

    dt = mybir.dt
    f32 = dt.float32
    f32r = dt.float32r
    AF = mybir.ActivationFunctionType
    OP = mybir.AluOpType
    AX = mybir.AxisListType

    nc = Bacc(None, target_bir_lowering=False)

    f_in = [nc.declare_dram_parameter(f"f{f+1}", [C, HW], f32, isOutput=False)
            for f in range(2)]
    w_in = [nc.declare_dram_parameter(f"w{f+1}t", [C, PQ], f32, isOutput=False)
            for f in range(2)]
    pb_in = [nc.declare_dram_parameter(f"pb{f+1}", [CL, 1], f32, isOutput=False)
             for f in range(2)]
    id_in = nc.declare_dram_parameter("ident", [128, 128], f32, isOutput=False)
    h_out = [nc.declare_dram_parameter(f"h{f+1}", [C, HW], f32, isOutput=True)
             for f in range(2)]
    s_out = [nc.declare_dram_parameter(f"s{f+1}", [C, HW], f32, isOutput=True)
             for f in range(2)]

    def r(ap):  # view as float32r (for PSUM transpose outs)
        return ap.bitcast(f32r)

    def v(ap):  # view a float32r tile as plain float32 (for DVE/gpsimd)
        return ap.bitcast(f32)

    with tile.TileContext(nc) as tc:
        with tc.tile_pool(name="const", bufs=1) as cst, \
             tc.tile_pool(name="state", bufs=1) as st, \
             tc.tile_pool(name="bd", bufs=1) as bdp, \
             tc.tile_pool(name="work", bufs=2) as wk, \
             tc.tile_pool(name="tiny", bufs=2) as ty, \
             tc.tile_pool(name="psb", bufs=2, space="PSUM") as psb, \
             tc.tile_pool(name="pss", bufs=4, space="PSUM") as pss:

            ident = cst.tile([128, 128], f32r, name="ident")
            nc.sync.dma_start(ident[:], r(id_in[:]))

            magic = cst.tile([128, 1], dt.int32, name="magic")
            nc.vector.memset(magic[:], 0x5f3759df)
            wT = [[cst.tile([128, PQ], f32r, tag=f"w{f}{cc}", name=f"w{f}{cc}")
                   for cc in range(2)] for f in range(2)]
            pb = [cst.tile([CL, 1], f32, tag=f"pb{f}", name=f"pb{f}")
                  for f in range(2)]
            for f in range(2):
                for cc in range(2):
                    nc.sync.dma_start(wT[f][cc][:],
                                      r(w_in[f][cc * 128:(cc + 1) * 128, :]))
                nc.sync.dma_start(pb[f][:], pb_in[f][:])

            fpN = [[st.tile([128, HW], f32r, tag=f"fpN{f}{cc}", name=f"fpN{f}{cc}")
                    for cc in range(2)] for f in range(2)]
            fpT = [[st.tile([128, C], f32, tag=f"fpT{f}{j}", name=f"fpT{f}{j}")
                    for j in range(8)] for f in range(2)]
            sT = [[st.tile([128, C], f32r, tag=f"sT{f}{j}", name=f"sT{f}{j}")
                   for j in range(8)] for f in range(2)]
            uT = [[st.tile([128, C], f32r, tag=f"uT{f}{j}", name=f"uT{f}{j}")
                   for j in range(8)] for f in range(2)]
            Bm = [bdp.tile([128, HW], f32r, tag=f"B{j}", name=f"B{j}")
                  for j in range(8)]
            Dm = [bdp.tile([128, HW], f32r, tag=f"D{j}", name=f"D{j}")
                  for j in range(8)]

            for f in range(2):
                for cc in range(2):
                    nc.sync.dma_start(fpN[f][cc][:],
                                      r(f_in[f][cc * 128:(cc + 1) * 128, :]))
            # initial fpT = transpose(fpN)
            for f in range(2):
                for j in range(8):
                    tp = pss.tile([128, 256], f32, tag="small", name="small")
                    for cc in range(2):
                        nc.tensor.transpose(
                            r(tp[:, cc * 128:(cc + 1) * 128]),
                            fpN[f][cc][:, j * 128:(j + 1) * 128], ident[:])
                    nc.vector.tensor_copy(fpT[f][j][:], tp[:])

            f_cl = [st.tile([CL, HW], f32r, tag=f"fcl{f}", name=f"fcl{f}")
                    for f in range(2)]

            for it in range(NITER):
                last = it == NITER - 1
                # ---- phase A: conv+proj matmul, softmax, s scaling ----
                for f in range(2):
                    q = psb.tile([128, HW], f32, tag="big", name="big")
                    for cc in range(2):
                        for ni in range(2):
                            nc.tensor.matmul(
                                q[0:PQ, ni * 512:(ni + 1) * 512],
                                wT[f][cc][:],
                                fpN[f][cc][:, ni * 512:(ni + 1) * 512],
                                start=(cc == 0), stop=(cc == 1))
                    negmax = ty.tile([1, 1], f32, tag="nm", name="nm")
                    nc.vector.tensor_reduce(negmax[:], q[32:33, :], axis=AX.X,
                                            op=OP.max, negate=True)
                    e_row = ty.tile([1, HW], f32, tag="erow", name="erow")
                    esum = ty.tile([1, 1], f32, tag="esum", name="esum")
                    nc.scalar.activation(e_row[:], q[32:33, :], AF.Exp,
                                         bias=negmax[:], accum_out=esum[:])
                    rcp = ty.tile([1, 1], f32, tag="rcp", name="rcp")
                    nc.vector.reciprocal(rcp[:], esum[:])
                    cs_row = ty.tile([1, HW], f32r, tag="csrow", name="csrow")
                    nc.vector.tensor_scalar(cs_row[:], e_row[:], rcp[:],
                                            None, op0=OP.mult)
                    # broadcast cs over 16 partitions, scale+bias proj rows
                    bc16 = ty.tile([CL, HW], f32, tag="bc16", name="bc16")
                    nc.gpsimd.partition_broadcast(bc16[:], v(cs_row[:]))
                    pcs = ty.tile([CL, HW], f32, tag="pcs", name="pcs")
                    nc.vector.scalar_tensor_tensor(
                        pcs[:], in0=q[0:16, :], scalar=1.0, in1=bc16[:],
                        op0=OP.mult, op1=OP.mult)
                    nc.vector.tensor_scalar(f_cl[f][:], pcs[:], pb[f][:],
                                            None, op0=OP.add)
                    # transposed conv column via PE, then exp + 1/sum
                    nmb = ty.tile([128, 1], f32, tag="nmb", name="nmb")
                    nc.gpsimd.partition_broadcast(nmb[:], negmax[:])
                    rcb = ty.tile([128, 1], f32, tag="rcb", name="rcb")
                    nc.gpsimd.partition_broadcast(rcb[:], rcp[:])
                    cT_ps = pss.tile([128, 16], f32, tag="small", name="small")
                    for j in range(8):
                        for cc in range(2):
                            nc.tensor.matmul(
                                cT_ps[:, 2 * j:2 * j + 2],
                                fpN[f][cc][:, j * 128:(j + 1) * 128],
                                wT[f][cc][:, 32:34],
                                start=(cc == 0), stop=(cc == 1))
                    csT = ty.tile([128, 8], f32, tag="csT", name="csT")
                    nc.scalar.activation(
                        csT[:], cT_ps[:].rearrange("p (j two) -> p j two",
                                                   two=2)[:, :, 0],
                        AF.Exp, bias=nmb[:])
                    for j in range(8):
                        nc.vector.tensor_scalar(sT[f][j][:], fpT[f][j][:],
                                                csT[:, j:j + 1], rcb[:],
                                                op0=OP.mult, op1=OP.mult)
                # ---- phase B: affinity matrices B and D with tanh ----
                for (dst, la, lb) in ((Bm, f_cl[0], f_cl[1]),
                                      (Dm, f_cl[1], f_cl[0])):
                    for mj in range(8):
                        ps = psb.tile([128, HW], f32, tag="big", name="big")
                        for ni in range(2):
                            nc.tensor.matmul(
                                ps[:, ni * 512:(ni + 1) * 512],
                                la[:, mj * 128:(mj + 1) * 128],
                                lb[:, ni * 512:(ni + 1) * 512],
                                start=True, stop=True)
                        nc.scalar.activation(dst[mj][:], ps[:], AF.Tanh)
                # ---- phase C: hats (transposed), l2 norm, residual ----
                for f in range(2):
                    mat = Bm if f == 0 else Dm
                    for mi in range(8):
                        hp = pss.tile([128, 256], f32, tag="small", name="small")
                        for jc in range(8):
                            nc.tensor.matmul(
                                hp[:],
                                mat[jc][:, mi * 128:(mi + 1) * 128],
                                sT[f][jc][:],
                                start=(jc == 0), stop=(jc == 7))
                        sqs = ty.tile([128, 256], f32, tag="sqs", name="sqs")
                        ssq = ty.tile([128, 1], f32, tag="ssq", name="ssq")
                        nc.scalar.activation(sqs[:], hp[:], AF.Square,
                                             accum_out=ssq[:])
                        # rsqrt via Quake initial guess + 1 Newton step
                        t0 = ty.tile([128, 1], f32, tag="t0", name="t0")
                        nc.vector.tensor_scalar(t0[:], ssq[:], 1e-24, None,
                                                op0=OP.max)
                        hx = ty.tile([128, 1], f32, tag="hx", name="hx")
                        nc.vector.tensor_scalar(hx[:], t0[:], -0.5, None,
                                                op0=OP.mult)
                        yi = ty.tile([128, 1], dt.int32, tag="yi", name="yi")
                        nc.vector.tensor_scalar(yi[:], t0[:].bitcast(dt.int32),
                                                1, None,
                                                op0=OP.arith_shift_right)
                        y0 = ty.tile([128, 1], f32, tag="y0", name="y0")
                        nc.vector.tensor_tensor(y0[:].bitcast(dt.int32),
                                                magic[:], yi[:],
                                                op=OP.subtract)
                        yc = y0
                        for nit in range(2):
                            yy = ty.tile([128, 1], f32, tag=f"yy{nit}",
                                         name=f"yy{nit}")
                            nc.vector.tensor_tensor(yy[:], yc[:], yc[:],
                                                    op=OP.mult)
                            w1 = ty.tile([128, 1], f32, tag=f"w1{nit}",
                                         name=f"w1{nit}")
                            nc.vector.scalar_tensor_tensor(
                                w1[:], in0=yy[:], scalar=1.0, in1=hx[:],
                                op0=OP.mult, op1=OP.mult)
                            w2 = ty.tile([128, 1], f32, tag=f"w2{nit}",
                                         name=f"w2{nit}")
                            nc.vector.tensor_scalar(w2[:], w1[:], 1.5, None,
                                                    op0=OP.add)
                            rn = ty.tile([128, 1], f32, tag=f"rn{nit}",
                                         name=f"rn{nit}")
                            nc.vector.tensor_tensor(rn[:], yc[:], w2[:],
                                                    op=OP.mult)
                            yc = rn
                        if not last:
                            nc.vector.scalar_tensor_tensor(
                                uT[f][mi][:], in0=hp[:], scalar=rn[:],
                                in1=fpT[f][mi][:], op0=OP.mult, op1=OP.add)
                            nc.vector.tensor_scalar_max(fpT[f][mi][:],
                                                        v(uT[f][mi][:]), 0.0)
                        else:
                            nc.vector.tensor_scalar(uT[f][mi][:], hp[:],
                                                    rn[:], None, op0=OP.mult)
                # ---- phase D: back-transpose ----
                if not last:
                    for f in range(2):
                        for cc in range(2):
                            tp = psb.tile([128, HW], f32, tag="big", name="big")
                            for mi in range(8):
                                nc.tensor.transpose(
                                    r(tp[:, mi * 128:(mi + 1) * 128]),
                                    uT[f][mi][:, cc * 128:(cc + 1) * 128],
                                    ident[:])
                            nc.vector.tensor_scalar_max(fpN[f][cc][:],
                                                        tp[:], 0.0)
                else:
                    for f in range(2):
                        for (src, dram) in ((uT[f], h_out[f]),
                                            (sT[f], s_out[f])):
                            for cc in range(2):
                                tp = psb.tile([128, HW], f32, tag="big",
                                              name="big")
                                for mi in range(8):
                                    nc.tensor.transpose(
                                        r(tp[:, mi * 128:(mi + 1) * 128]),
                                        src[mi][:, cc * 128:(cc + 1) * 128],
                                        ident[:])
                                ot = wk.tile([128, HW], f32, tag="out",
                                             name="out")
                                nc.vector.tensor_copy(ot[:], tp[:])
                                nc.sync.dma_start(
                                    dram[cc * 128:(cc + 1) * 128, :], ot[:])

    nc.compile()
    return nc


def _get_nc():
    if "nc" not in _CACHE:
        _CACHE["nc"] = _build()
    return _CACHE["nc"]


def make_in_maps(f1, f2, pw1, pb1, pw2, pb2, cw1, cb1, cw2, cb2):
    def packw(cw, pw):
        w = np.zeros((PQ, C), np.float32)
        w[0:CL] = pw
        w[32] = cw[0]
        return np.ascontiguousarray(w.T).astype(np.float32)  # [C, 48]
    w1t = packw(cw1, pw1)
    w2t = packw(cw2, pw2)
    pb1c = np.ascontiguousarray(pb1.reshape(CL, 1)).astype(np.float32)
    pb2c = np.ascontiguousarray(pb2.reshape(CL, 1)).astype(np.float32)
    ident = np.eye(128, dtype=np.float32)
    in_maps = []
    for n in range(f1.shape[0]):
        in_maps.append({
            "f1": np.ascontiguousarray(f1[n].reshape(C, HW)).astype(np.float32),
            "f2": np.ascontiguousarray(f2[n].reshape(C, HW)).astype(np.float32),
            "w1t": w1t, "w2t": w2t, "pb1": pb1c, "pb2": pb2c, "ident": ident,
        })
    return in_maps


def kernel(f1, f2, pw1, pb1, pw2, pb2, cw1, cb1, cw2, cb2):
    from concourse.bass_utils import run_bass_kernel_spmd

    nc = _get_nc()
    N = f1.shape[0]
    assert N == NCORES
    in_maps = make_in_maps(f1, f2, pw1, pb1, pw2, pb2, cw1, cb1, cw2, cb2)
    res = run_bass_kernel_spmd(nc, in_maps, core_ids=list(range(NCORES)))
    H, W = 32, 32
    outs = []
    for name in ("h1", "h2", "s1", "s2"):
        outs.append(np.stack([res.results[n][name] for n in range(N)])
                    .reshape(N, C, H, W))
    return tuple(outs)
